# revision 4
# baseline (speedup 1.0000x reference)
"""CameraToBEV Trainium2 kernel.

Pipeline (per the reference module): feature head conv(3->128,3x3) -> ReLU ->
conv(128->256,3x3), then a static perspective gather into a [B,256,200,200]
BEV grid.  The depth head is dead code (its output is unused) and is skipped.

Key structural facts exploited (all static, derived from the module config):
  * only 99 BEV rows (101..199) have any valid cell; each BEV row reads from a
    single feature row; only 22 distinct feature rows are ever gathered;
  * per BEV row the gather column map u(j) is static, so the gather is a
    matmul against a static 0/1 selection matrix (zero columns produce the
    masked zeros exactly);
  * all convolution arithmetic is done as bf16 hi/lo decomposed matmuls
    (3 terms), which reproduces fp32 precision to ~1e-5.

Sharding: 8 cores = batch (4) x BEV-column-half (2).  Each core computes
feature columns for its half (128 wide + halo), all 22 feature rows, and the
99x100 BEV quadrant for its (b, half).  SPMD: one program, per-core inputs.
"""

import numpy as np
import ml_dtypes

_BF16 = ml_dtypes.bfloat16

# ---------------- static module config ----------------
_B, _CIN, _H, _W = 4, 3, 256, 256
_BEV_H = _BEV_W = 200
_C1, _C2 = 128, 256
_JH = _BEV_W // 2          # 100 BEV columns per core
_W1COLS = 131              # conv1 output cols per core; local L <-> global wh0-1+L
_IMGCOLS = 133             # image cols per core;        local c <-> global wh0-2+c
_NEG = np.float32(-1e30)


def _compute_static():
    """Static gather maps.  MUST replicate the reference's float32 jax math
    bit-exactly (trunc boundaries differ from float64)."""
    import jax
    import jax.numpy as jnp

    cpu = jax.devices("cpu")[0]
    with jax.default_device(cpu):
        y = jnp.linspace(-50.0, 50.0, _BEV_H)
        x = jnp.linspace(-50.0, 50.0, _BEV_W)
        depth = y / 100.0 * 100.0
        u = jnp.trunc(_W / 2 + x[None, :] / depth[:, None] * 50.0).astype(jnp.int32)
        v = jnp.trunc(_H / 2 - 1.5 / depth * 50.0).astype(jnp.int32)
        mask = (y[:, None] > 0.1) & (u >= 0) & (u < _W) & \
               (v[:, None] >= 0) & (v[:, None] < _H)
    U = np.asarray(u).astype(np.int64)
    V = np.asarray(v).astype(np.int64)
    MASK = np.asarray(mask)

    valid_rows = np.where(MASK.any(axis=1))[0]
    I0 = int(valid_rows.min())
    used_v = sorted(set(int(V[i]) for i in valid_rows))

    # conv1 rows needed (feature rows +-1), as packed list
    need1 = sorted(set(r for vv in used_v for r in (vv - 1, vv, vv + 1)))
    r1_idx = {r: k for k, r in enumerate(need1)}

    # consecutive BEV rows sharing one feature row v
    groups = []
    cur_v = None
    for i in range(I0, _BEV_H):
        vv = int(V[i])
        if cur_v is not None and vv == cur_v:
            groups[-1] = (cur_v, groups[-1][1], groups[-1][2] + 1)
        else:
            groups.append((vv, i, 1))
            cur_v = vv
    v_idx = {vv: k for k, vv in enumerate(used_v)}
    return dict(U=U, V=V, MASK=MASK, I0=I0, USED_V=used_v, R1_ROWS=need1,
                R1_IDX=r1_idx, GROUPS=groups, V_IDX=v_idx)


_ST = None


def _static():
    global _ST
    if _ST is None:
        _ST = _compute_static()
    return _ST


# ---------------- host-side input prep ----------------

def _split_hilo(x):
    x = np.ascontiguousarray(x, dtype=np.float32)
    hi = x.astype(_BF16)
    lo = (x - hi.astype(np.float32)).astype(_BF16)
    return hi, lo


def _build_S(st, jhalf, wh0):
    I0 = st["I0"]
    n_rows = _BEV_H - I0
    S = np.zeros((128, n_rows * _JH), dtype=np.float32)
    U, MASK = st["U"], st["MASK"]
    for i in range(I0, _BEV_H):
        base = (i - I0) * _JH
        for j in range(_JH):
            jj = jhalf * _JH + j
            if MASK[i, jj]:
                S[int(U[i, jj]) - wh0, base + j] = 1.0
    return S.astype(_BF16)


def _host_prep(st, images, fw1, fb1, fw2, fb2):
    """Returns list of 8 in_maps (core = b*2 + jhalf)."""
    R1_ROWS = st["R1_ROWS"]
    nr1 = len(R1_ROWS)
    img_rows = sorted(set(r for r1 in R1_ROWS for r in (r1 - 1, r1, r1 + 1)))
    img_idx = {r: k for k, r in enumerate(img_rows)}

    # weights (shared across cores)
    w1p = np.zeros((27, _C1), np.float32)
    for dy in range(3):
        for dx in range(3):
            for ci in range(3):
                w1p[(dy * 3 + dx) * 3 + ci, :] = fw1[:, ci, dy, dx]
    w1_hi, w1_lo = _split_hilo(w1p)

    w2t = np.zeros((9, _C1, _C2), np.float32)
    for dy in range(3):
        for dx in range(3):
            w2t[dy * 3 + dx] = fw2[:, :, dy, dx].T
    w2_hi, w2_lo = _split_hilo(w2t.transpose(1, 0, 2).copy())  # [128, 9, 256]

    fb1h, fb1l = _split_hilo(fb1)
    b1_lhsT = np.ascontiguousarray(np.stack(
        [fb1h.astype(np.float32), fb1l.astype(np.float32),
         np.ones(_C1, np.float32)]).astype(_BF16))
    fb2h, fb2l = _split_hilo(fb2)
    b2_lhsT = np.ones((2, _C1), np.float32).astype(_BF16)
    b2_rhs = np.ascontiguousarray(np.stack(
        [fb2h.astype(np.float32), fb2l.astype(np.float32)]).astype(_BF16))

    in_maps = []
    S_cache = {}
    for b in range(_B):
        for jh in range(2):
            wh0 = jh * 128
            # padded image slice [3, nimg, 133], global cols wh0-2 .. wh0+130
            imgp = np.zeros((3, len(img_rows), _IMGCOLS), np.float32)
            g0 = wh0 - 2
            lo = max(0, -g0)
            hi = min(_IMGCOLS, _W - g0)
            for k, r in enumerate(img_rows):
                imgp[:, k, lo:hi] = images[b, :, r, g0 + lo:g0 + hi]
            img_hi, img_lo = _split_hilo(imgp)

            # host-side im2col: [27, nr1, 131]
            def _im2col(src):
                G = np.zeros((27, nr1, _W1COLS), src.dtype)
                for rr, r1 in enumerate(R1_ROWS):
                    for dy in range(3):
                        ip = img_idx[r1 + dy - 1]
                        for dx in range(3):
                            for ci in range(3):
                                G[(dy * 3 + dx) * 3 + ci, rr, :] = \
                                    src[ci, ip, dx:dx + _W1COLS]
                return G

            cb = np.zeros((_W1COLS,), np.float32)
            cb[0 if jh == 0 else 129] = _NEG  # zero-pad column of R1 via relu(-inf)
            b1_rhs = np.ascontiguousarray(np.stack(
                [np.ones(_W1COLS, np.float32), np.ones(_W1COLS, np.float32),
                 cb]).astype(_BF16))

            if jh not in S_cache:
                S_cache[jh] = _build_S(st, jh, wh0)

            in_maps.append({
                "g1_hi": np.ascontiguousarray(_im2col(img_hi)),
                "g1_lo": np.ascontiguousarray(_im2col(img_lo)),
                "w1_hi": w1_hi, "w1_lo": w1_lo,
                "b1_lhsT": b1_lhsT, "b1_rhs": b1_rhs,
                "w2_hi": w2_hi, "w2_lo": w2_lo,
                "b2_lhsT": b2_lhsT, "b2_rhs": b2_rhs,
                "S": S_cache[jh],
            })
    return in_maps


# ---------------- bass program ----------------

def _build_nc(st):
    import concourse.bass as bass
    import concourse.mybir as mybir
    import concourse.tile as tile
    from concourse import bacc
    from contextlib import ExitStack

    bf = mybir.dt.bfloat16
    f32 = mybir.dt.float32

    R1_ROWS, R1_IDX = st["R1_ROWS"], st["R1_IDX"]
    USED_V, V_IDX, GROUPS, I0 = st["USED_V"], st["V_IDX"], st["GROUPS"], st["I0"]
    nr1 = len(R1_ROWS)
    nv = len(USED_V)
    n_rows = _BEV_H - I0
    scols = n_rows * _JH

    nc = bacc.Bacc(None, target_bir_lowering=False)

    g1_hi = nc.dram_tensor("g1_hi", [27, nr1, _W1COLS], bf, kind="ExternalInput")
    g1_lo = nc.dram_tensor("g1_lo", [27, nr1, _W1COLS], bf, kind="ExternalInput")
    w1_hi = nc.dram_tensor("w1_hi", [27, _C1], bf, kind="ExternalInput")
    w1_lo = nc.dram_tensor("w1_lo", [27, _C1], bf, kind="ExternalInput")
    b1_lhsT = nc.dram_tensor("b1_lhsT", [3, _C1], bf, kind="ExternalInput")
    b1_rhs = nc.dram_tensor("b1_rhs", [3, _W1COLS], bf, kind="ExternalInput")
    w2_hi = nc.dram_tensor("w2_hi", [_C1, 9, _C2], bf, kind="ExternalInput")
    w2_lo = nc.dram_tensor("w2_lo", [_C1, 9, _C2], bf, kind="ExternalInput")
    b2_lhsT = nc.dram_tensor("b2_lhsT", [2, _C1], bf, kind="ExternalInput")
    b2_rhs = nc.dram_tensor("b2_rhs", [2, _C2], bf, kind="ExternalInput")
    S = nc.dram_tensor("S", [128, scols], bf, kind="ExternalInput")
    out = nc.dram_tensor("out", [_C2, _BEV_H, _JH], f32, kind="ExternalOutput")

    with tile.TileContext(nc) as tc:
        with ExitStack() as ctx:
            singles = ctx.enter_context(tc.tile_pool(name="singles", bufs=1))
            tmp = ctx.enter_context(tc.tile_pool(name="tmp", bufs=3))
            stgp = ctx.enter_context(tc.tile_pool(name="stg", bufs=4))
            p1 = ctx.enter_context(tc.tile_pool(name="p1", bufs=2, space="PSUM"))
            p2 = ctx.enter_context(tc.tile_pool(name="p2", bufs=2, space="PSUM"))
            pg = ctx.enter_context(tc.tile_pool(name="pg", bufs=3, space="PSUM"))

            def load(name, dram, shape, dt):
                t = singles.tile(shape, dt, tag=name)
                nc.sync.dma_start(out=t[:], in_=dram[:])
                return t

            g1h_t = load("g1h", g1_hi, [27, nr1, _W1COLS], bf)
            g1l_t = load("g1l", g1_lo, [27, nr1, _W1COLS], bf)
            w1h_t = load("w1h", w1_hi, [27, _C1], bf)
            w1l_t = load("w1l", w1_lo, [27, _C1], bf)
            b1l_t = load("b1l", b1_lhsT, [3, _C1], bf)
            b1r_t = load("b1r", b1_rhs, [3, _W1COLS], bf)
            w2h_t = load("w2h", w2_hi, [_C1, 9, _C2], bf)
            w2l_t = load("w2l", w2_lo, [_C1, 9, _C2], bf)
            b2l_t = load("b2l", b2_lhsT, [2, _C1], bf)
            b2r_t = load("b2r", b2_rhs, [2, _C2], bf)
            s_t = load("S", S, [128, scols], bf)

            r1h = singles.tile([_C1, nr1, _W1COLS], bf, tag="r1h")
            r1l = singles.tile([_C1, nr1, _W1COLS], bf, tag="r1l")
            fth = singles.tile([128, nv, _C2], bf, tag="fth")
            ftl = singles.tile([128, nv, _C2], bf, tag="ftl")

            # ---- conv1 ----
            for rr in range(nr1):
                ps = p1.tile([_C1, _W1COLS], f32, tag="ps1")
                nc.tensor.matmul(ps[:], w1h_t[:], g1h_t[:, rr, :], start=True, stop=False)
                nc.tensor.matmul(ps[:], w1h_t[:], g1l_t[:, rr, :], start=False, stop=False)
                nc.tensor.matmul(ps[:], w1l_t[:], g1h_t[:, rr, :], start=False, stop=False)
                nc.tensor.matmul(ps[:], b1l_t[:], b1r_t[:], start=False, stop=True)
                r1row = tmp.tile([_C1, _W1COLS], f32, tag="r1row")
                nc.scalar.activation(r1row[:], ps[:],
                                     mybir.ActivationFunctionType.Relu)
                nc.vector.tensor_copy(r1h[:, rr, :], r1row[:])
                nc.vector.tensor_sub(r1l[:, rr, :], r1row[:], r1h[:, rr, :])

            # ---- conv2 at the 22 needed feature rows ----
            for vi, vv in enumerate(USED_V):
                ps = p2.tile([128, _C2], f32, tag="ps2")
                first = True
                for dy in range(3):
                    rr = R1_IDX[vv + dy - 1]
                    for dx in range(3):
                        tap = dy * 3 + dx
                        lh = r1h[:, rr, dx:dx + 128]
                        ll = r1l[:, rr, dx:dx + 128]
                        rh = w2h_t[:, tap, :]
                        rl = w2l_t[:, tap, :]
                        nc.tensor.matmul(ps[:], lh, rh, start=first, stop=False)
                        first = False
                        nc.tensor.matmul(ps[:], ll, rh, start=False, stop=False)
                        nc.tensor.matmul(ps[:], lh, rl, start=False, stop=False)
                nc.tensor.matmul(ps[:], b2l_t[:], b2r_t[:], start=False, stop=True)
                nc.vector.tensor_copy(fth[:, vi, :], ps[:])
                nc.vector.tensor_sub(ftl[:, vi, :], ps[:], fth[:, vi, :])

            # ---- gather (selection matmuls), 5 BEV rows (=500 cols) per chunk ----
            for (vv, ist, nrow) in GROUPS:
                vi = V_IDX[vv]
                done = 0
                while done < nrow:
                    step = min(5, nrow - done)
                    c0 = (ist - I0 + done) * _JH
                    ncols = step * _JH
                    for ch in range(2):
                        psg = pg.tile([128, 5 * _JH], f32, tag="psg")
                        lh = fth[:, vi, ch * 128:(ch + 1) * 128]
                        ll = ftl[:, vi, ch * 128:(ch + 1) * 128]
                        nc.tensor.matmul(psg[:, 0:ncols], lh, s_t[:, c0:c0 + ncols],
                                         start=True, stop=False)
                        nc.tensor.matmul(psg[:, 0:ncols], ll, s_t[:, c0:c0 + ncols],
                                         start=False, stop=True)
                        stg = stgp.tile([128, 5, _JH], f32, tag="stg")
                        nc.vector.tensor_copy(
                            stg[:, 0:step, :].rearrange("p a b -> p (a b)"),
                            psg[:, 0:ncols])
                        nc.sync.dma_start(
                            out=out[ch * 128:(ch + 1) * 128,
                                    ist + done:ist + done + step, :],
                            in_=stg[:, 0:step, :])
                    done += step

    nc.compile()
    return nc


_NC_CACHE = None


def kernel(images, dw1, db1, dw2, db2, fw1, fb1, fw2, fb2):
    global _NC_CACHE
    from concourse.bass_utils import run_bass_kernel_spmd

    images = np.asarray(images, dtype=np.float32)
    fw1 = np.asarray(fw1, dtype=np.float32)
    fb1 = np.asarray(fb1, dtype=np.float32)
    fw2 = np.asarray(fw2, dtype=np.float32)
    fb2 = np.asarray(fb2, dtype=np.float32)

    st = _static()
    in_maps = _host_prep(st, images, fw1, fb1, fw2, fb2)
    if _NC_CACHE is None:
        _NC_CACHE = _build_nc(st)
    nc = _NC_CACHE

    res = run_bass_kernel_spmd(nc, in_maps, core_ids=list(range(8)))

    full = np.zeros((_B, _C2, _BEV_H, _BEV_W), np.float32)
    for b in range(_B):
        for jh in range(2):
            r = res.results[b * 2 + jh]["out"]
            full[b, :, :, jh * _JH:(jh + 1) * _JH] = r
    return full


# revision 8
# speedup vs baseline: 1.2261x; 1.2261x over previous
"""CameraToBEV Trainium2 kernel.

Pipeline (per the reference module): feature head conv(3->128,3x3) -> ReLU ->
conv(128->256,3x3), then a static perspective gather into a [B,256,200,200]
BEV grid.  The depth head is dead code (its output is unused) and is skipped.

Key structural facts exploited (all static, derived from the module config):
  * only 99 BEV rows (101..199) have any valid cell; each BEV row reads from a
    single feature row; only 22 distinct feature rows are ever gathered;
  * per BEV row the gather column map u(j) is static, so the gather is a
    matmul against a static 0/1 selection matrix (zero columns produce the
    masked zeros exactly);
  * all convolution arithmetic is done as bf16 hi/lo decomposed matmuls
    (3 terms), which reproduces fp32 precision to ~1e-5.

Sharding: 8 cores = batch (4) x BEV-column-half (2).  Each core computes
feature columns for its half (128 wide + halo), all 22 feature rows, and the
99x100 BEV quadrant for its (b, half).  SPMD: one program, per-core inputs.
"""

import numpy as np
import ml_dtypes

_BF16 = ml_dtypes.bfloat16

# ---------------- static module config ----------------
_B, _CIN, _H, _W = 4, 3, 256, 256
_BEV_H = _BEV_W = 200
_C1, _C2 = 128, 256
_JH = _BEV_W // 2          # 100 BEV columns per core
_W1COLS = 131              # conv1 output cols per core; local L <-> global wh0-1+L
_IMGCOLS = 133             # image cols per core;        local c <-> global wh0-2+c
_NEG = np.float32(-1e30)


def _compute_static():
    """Static gather maps.  MUST replicate the reference's float32 jax math
    bit-exactly (trunc boundaries differ from float64)."""
    import jax
    import jax.numpy as jnp

    cpu = jax.devices("cpu")[0]
    with jax.default_device(cpu):
        y = jnp.linspace(-50.0, 50.0, _BEV_H)
        x = jnp.linspace(-50.0, 50.0, _BEV_W)
        depth = y / 100.0 * 100.0
        u = jnp.trunc(_W / 2 + x[None, :] / depth[:, None] * 50.0).astype(jnp.int32)
        v = jnp.trunc(_H / 2 - 1.5 / depth * 50.0).astype(jnp.int32)
        mask = (y[:, None] > 0.1) & (u >= 0) & (u < _W) & \
               (v[:, None] >= 0) & (v[:, None] < _H)
    U = np.asarray(u).astype(np.int64)
    V = np.asarray(v).astype(np.int64)
    MASK = np.asarray(mask)

    valid_rows = np.where(MASK.any(axis=1))[0]
    I0 = int(valid_rows.min())
    used_v = sorted(set(int(V[i]) for i in valid_rows))

    # conv1 rows needed (feature rows +-1), as packed list
    need1 = sorted(set(r for vv in used_v for r in (vv - 1, vv, vv + 1)))
    r1_idx = {r: k for k, r in enumerate(need1)}

    # consecutive BEV rows sharing one feature row v
    groups = []
    cur_v = None
    for i in range(I0, _BEV_H):
        vv = int(V[i])
        if cur_v is not None and vv == cur_v:
            groups[-1] = (cur_v, groups[-1][1], groups[-1][2] + 1)
        else:
            groups.append((vv, i, 1))
            cur_v = vv
    v_idx = {vv: k for k, vv in enumerate(used_v)}
    return dict(U=U, V=V, MASK=MASK, I0=I0, USED_V=used_v, R1_ROWS=need1,
                R1_IDX=r1_idx, GROUPS=groups, V_IDX=v_idx)


_ST = None


def _static():
    global _ST
    if _ST is None:
        _ST = _compute_static()
    return _ST


# ---------------- host-side input prep ----------------

def _split_hilo(x):
    x = np.ascontiguousarray(x, dtype=np.float32)
    hi = x.astype(_BF16)
    lo = (x - hi.astype(np.float32)).astype(_BF16)
    return hi, lo


def _build_S(st, jhalf, wh0):
    I0 = st["I0"]
    n_rows = _BEV_H - I0
    S = np.zeros((128, n_rows * _JH), dtype=np.float32)
    U, MASK = st["U"], st["MASK"]
    for i in range(I0, _BEV_H):
        base = (i - I0) * _JH
        for j in range(_JH):
            jj = jhalf * _JH + j
            if MASK[i, jj]:
                S[int(U[i, jj]) - wh0, base + j] = 1.0
    return S.astype(_BF16)


def _host_prep(st, images, fw1, fb1, fw2, fb2):
    """Returns list of 8 in_maps (core = b*2 + jhalf)."""
    R1_ROWS = st["R1_ROWS"]
    nr1 = len(R1_ROWS)
    img_rows = sorted(set(r for r1 in R1_ROWS for r in (r1 - 1, r1, r1 + 1)))
    img_idx = {r: k for k, r in enumerate(img_rows)}

    # weights (shared across cores)
    w1p = np.zeros((27, _C1), np.float32)
    for dy in range(3):
        for dx in range(3):
            for ci in range(3):
                w1p[(dy * 3 + dx) * 3 + ci, :] = fw1[:, ci, dy, dx]
    w1_hi, w1_lo = _split_hilo(w1p)
    fb1h, fb1l = _split_hilo(fb1)
    # stacked conv1 lhsT [84, 128]: [w1_hi; w1_hi; w1_lo; fb1_hi; fb1_lo; ones]
    w1_stack = np.concatenate([
        w1_hi.astype(np.float32), w1_hi.astype(np.float32),
        w1_lo.astype(np.float32),
        fb1h.astype(np.float32)[None, :], fb1l.astype(np.float32)[None, :],
        np.ones((1, _C1), np.float32)]).astype(_BF16)

    w2t = np.zeros((9, _C1, _C2), np.float32)
    for dy in range(3):
        for dx in range(3):
            w2t[dy * 3 + dx] = fw2[:, :, dy, dx].T
    w2_hi, w2_lo = _split_hilo(w2t.transpose(1, 0, 2).copy())  # [128, 9, 256]

    fb2h, fb2l = _split_hilo(fb2)
    b2_lhsT = np.ones((2, _C1), np.float32).astype(_BF16)
    b2_rhs = np.ascontiguousarray(np.stack(
        [fb2h.astype(np.float32), fb2l.astype(np.float32)]).astype(_BF16))

    in_maps = []
    S_cache = {}
    for b in range(_B):
        for jh in range(2):
            wh0 = jh * 128
            # padded image slice [3, nimg, 133], global cols wh0-2 .. wh0+130
            imgp = np.zeros((3, len(img_rows), _IMGCOLS), np.float32)
            g0 = wh0 - 2
            lo = max(0, -g0)
            hi = min(_IMGCOLS, _W - g0)
            for k, r in enumerate(img_rows):
                imgp[:, k, lo:hi] = images[b, :, r, g0 + lo:g0 + hi]
            img_hi, img_lo = _split_hilo(imgp)

            # host-side im2col: [27, nr1, 131]
            def _im2col(src):
                G = np.zeros((27, nr1, _W1COLS), src.dtype)
                for rr, r1 in enumerate(R1_ROWS):
                    for dy in range(3):
                        ip = img_idx[r1 + dy - 1]
                        for dx in range(3):
                            for ci in range(3):
                                G[(dy * 3 + dx) * 3 + ci, rr, :] = \
                                    src[ci, ip, dx:dx + _W1COLS]
                return G

            cb = np.zeros((_W1COLS,), np.float32)
            cb[0 if jh == 0 else 129] = _NEG  # zero-pad column of R1 via relu(-inf)
            ones_r = np.ones((nr1, _W1COLS), np.float32)
            # stacked conv1 rhs [84, nr1, 131]: [g1_hi; g1_lo; g1_hi; 1; colbias; 1... ]
            # rows 81..83 pair with lhsT rows [fb1_hi; fb1_lo; ones]:
            #   row81 = ones (adds fb1_hi), row82 = ones (adds fb1_lo),
            #   row83 = colbias (ones row in lhsT -> adds colbias per column)
            g1h = _im2col(img_hi).astype(np.float32)
            g1l = _im2col(img_lo).astype(np.float32)
            g1_stack = np.concatenate([
                g1h, g1l, g1h,
                ones_r[None], ones_r[None],
                np.broadcast_to(cb[None, None, :], (1, nr1, _W1COLS)),
            ]).astype(_BF16)

            if jh not in S_cache:
                S_cache[jh] = _build_S(st, jh, wh0)

            in_maps.append({
                "g1_stack": np.ascontiguousarray(g1_stack),
                "w1_stack": w1_stack,
                "w2_hi": w2_hi, "w2_lo": w2_lo,
                "b2_lhsT": b2_lhsT, "b2_rhs": b2_rhs,
                "S": S_cache[jh],
            })
    return in_maps


# ---------------- bass program ----------------

def _build_nc(st):
    import concourse.bass as bass
    import concourse.mybir as mybir
    import concourse.tile as tile
    from concourse import bacc
    from contextlib import ExitStack

    bf = mybir.dt.bfloat16
    f32 = mybir.dt.float32

    R1_ROWS, R1_IDX = st["R1_ROWS"], st["R1_IDX"]
    USED_V, V_IDX, GROUPS, I0 = st["USED_V"], st["V_IDX"], st["GROUPS"], st["I0"]
    nr1 = len(R1_ROWS)
    nv = len(USED_V)
    n_rows = _BEV_H - I0
    scols = n_rows * _JH

    nc = bacc.Bacc(None, target_bir_lowering=False)

    g1_stack = nc.dram_tensor("g1_stack", [84, nr1, _W1COLS], bf, kind="ExternalInput")
    w1_stack = nc.dram_tensor("w1_stack", [84, _C1], bf, kind="ExternalInput")
    w2_hi = nc.dram_tensor("w2_hi", [_C1, 9, _C2], bf, kind="ExternalInput")
    w2_lo = nc.dram_tensor("w2_lo", [_C1, 9, _C2], bf, kind="ExternalInput")
    b2_lhsT = nc.dram_tensor("b2_lhsT", [2, _C1], bf, kind="ExternalInput")
    b2_rhs = nc.dram_tensor("b2_rhs", [2, _C2], bf, kind="ExternalInput")
    S = nc.dram_tensor("S", [128, scols], bf, kind="ExternalInput")
    out = nc.dram_tensor("out", [_C2, _BEV_H, _JH], f32, kind="ExternalOutput")

    with tile.TileContext(nc) as tc:
        with ExitStack() as ctx:
            singles = ctx.enter_context(tc.tile_pool(name="singles", bufs=1))
            tmp = ctx.enter_context(tc.tile_pool(name="tmp", bufs=3))
            stgp = ctx.enter_context(tc.tile_pool(name="stg", bufs=8))

            def load(name, dram, shape, dt, split=1):
                t = singles.tile(shape, dt, tag=name)
                if split == 1:
                    nc.sync.dma_start(out=t[:], in_=dram[:])
                else:
                    n = shape[1]
                    step = (n + split - 1) // split
                    for a in range(0, n, step):
                        b = min(a + step, n)
                        nc.sync.dma_start(out=t[:, a:b], in_=dram[:, a:b])
                return t

            g1_t = load("g1", g1_stack, [84, nr1, _W1COLS], bf, split=4)
            w1_t = load("w1", w1_stack, [84, _C1], bf)
            w2h_t = load("w2h", w2_hi, [_C1, 9, _C2], bf)
            w2l_t = load("w2l", w2_lo, [_C1, 9, _C2], bf)
            b2l_t = load("b2l", b2_lhsT, [2, _C1], bf)
            b2r_t = load("b2r", b2_rhs, [2, _C2], bf)
            s_t = load("S", S, [128, scols], bf, split=4)

            r1h = singles.tile([_C1, nr1, _W1COLS], bf, tag="r1h")
            r1l = singles.tile([_C1, nr1, _W1COLS], bf, tag="r1l")
            fth = singles.tile([128, nv, _C2], bf, tag="fth")
            ftl = singles.tile([128, nv, _C2], bf, tag="ftl")

            with ExitStack() as cctx:
                p1 = cctx.enter_context(tc.tile_pool(name="p1", bufs=3, space="PSUM"))
                p2 = cctx.enter_context(tc.tile_pool(name="p2", bufs=3, space="PSUM"))

                # ---- conv1: single K=84 stacked matmul per row ----
                for rr in range(nr1):
                    ps = p1.tile([_C1, _W1COLS], f32, tag="ps1")
                    nc.tensor.matmul(ps[:], w1_t[:], g1_t[:, rr, :],
                                     start=True, stop=True)
                    r1row = tmp.tile([_C1, _W1COLS], f32, tag="r1row")
                    nc.scalar.activation(r1row[:], ps[:],
                                         mybir.ActivationFunctionType.Relu)
                    nc.vector.tensor_copy(r1h[:, rr, :], r1row[:])
                    nc.vector.tensor_sub(r1l[:, rr, :], r1row[:], r1h[:, rr, :])

                # ---- conv2 at the 22 needed feature rows ----
                for vi, vv in enumerate(USED_V):
                    ps = p2.tile([128, _C2], f32, tag="ps2")
                    first = True
                    for dy in range(3):
                        rr = R1_IDX[vv + dy - 1]
                        for dx in range(3):
                            tap = dy * 3 + dx
                            lh = r1h[:, rr, dx:dx + 128]
                            ll = r1l[:, rr, dx:dx + 128]
                            rh = w2h_t[:, tap, :]
                            rl = w2l_t[:, tap, :]
                            nc.tensor.matmul(ps[:], lh, rh, start=first, stop=False)
                            first = False
                            nc.tensor.matmul(ps[:], ll, rh, start=False, stop=False)
                            nc.tensor.matmul(ps[:], lh, rl, start=False, stop=False)
                    nc.tensor.matmul(ps[:], b2l_t[:], b2r_t[:], start=False, stop=True)
                    nc.scalar.copy(fth[:, vi, :], ps[:])
                    nc.vector.tensor_sub(ftl[:, vi, :], ps[:], fth[:, vi, :])

            # ---- gather (selection matmuls), 5 BEV rows (=500 cols) per chunk ----
            with ExitStack() as gctx:
                pg = gctx.enter_context(tc.tile_pool(name="pg", bufs=6, space="PSUM"))
                copy_idx = 0
                for (vv, ist, nrow) in GROUPS:
                    vi = V_IDX[vv]
                    done = 0
                    while done < nrow:
                        step = min(5, nrow - done)
                        c0 = (ist - I0 + done) * _JH
                        ncols = step * _JH
                        for ch in range(2):
                            psg = pg.tile([128, 5 * _JH], f32, tag="psg")
                            lh = fth[:, vi, ch * 128:(ch + 1) * 128]
                            ll = ftl[:, vi, ch * 128:(ch + 1) * 128]
                            nc.tensor.matmul(psg[:, 0:ncols], lh,
                                             s_t[:, c0:c0 + ncols],
                                             start=True, stop=False)
                            nc.tensor.matmul(psg[:, 0:ncols], ll,
                                             s_t[:, c0:c0 + ncols],
                                             start=False, stop=True)
                            stg = stgp.tile([128, 5, _JH], f32, tag="stg")
                            dst = stg[:, 0:step, :].rearrange("p a b -> p (a b)")
                            if copy_idx % 2 == 0:
                                nc.vector.tensor_copy(dst, psg[:, 0:ncols])
                            else:
                                nc.scalar.copy(dst, psg[:, 0:ncols])
                            copy_idx += 1
                            nc.sync.dma_start(
                                out=out[ch * 128:(ch + 1) * 128,
                                        ist + done:ist + done + step, :],
                                in_=stg[:, 0:step, :])
                        done += step

    nc.compile()
    return nc


_NC_CACHE = None


def kernel(images, dw1, db1, dw2, db2, fw1, fb1, fw2, fb2):
    global _NC_CACHE
    from concourse.bass_utils import run_bass_kernel_spmd

    images = np.asarray(images, dtype=np.float32)
    fw1 = np.asarray(fw1, dtype=np.float32)
    fb1 = np.asarray(fb1, dtype=np.float32)
    fw2 = np.asarray(fw2, dtype=np.float32)
    fb2 = np.asarray(fb2, dtype=np.float32)

    st = _static()
    in_maps = _host_prep(st, images, fw1, fb1, fw2, fb2)
    if _NC_CACHE is None:
        _NC_CACHE = _build_nc(st)
    nc = _NC_CACHE

    res = run_bass_kernel_spmd(nc, in_maps, core_ids=list(range(8)))

    full = np.zeros((_B, _C2, _BEV_H, _BEV_W), np.float32)
    for b in range(_B):
        for jh in range(2):
            r = res.results[b * 2 + jh]["out"]
            full[b, :, :, jh * _JH:(jh + 1) * _JH] = r
    return full


# revision 16
# speedup vs baseline: 1.3897x; 1.1334x over previous
"""CameraToBEV Trainium2 kernel.

Pipeline (per the reference module): feature head conv(3->128,3x3) -> ReLU ->
conv(128->256,3x3), then a static perspective gather into a [B,256,200,200]
BEV grid.  The depth head is dead code (its output is unused) and is skipped.

Key structural facts exploited (all static, derived from the module config):
  * only 99 BEV rows (101..199) have any valid cell; each BEV row reads from a
    single feature row; only 22 distinct feature rows are ever gathered;
  * per BEV row the gather column map u(j) is static, so the gather is a
    matmul against a static 0/1 selection matrix (zero columns produce the
    masked zeros exactly);
  * all convolution arithmetic is done as bf16 hi/lo decomposed matmuls
    (3 terms), which reproduces fp32 precision to ~1e-5.

Sharding: 8 cores = batch (4) x BEV-column-half (2).  Each core computes
feature columns for its half (128 wide + halo), all 22 feature rows, and the
99x100 BEV quadrant for its (b, half).  SPMD: one program, per-core inputs.
"""

import numpy as np
import ml_dtypes

_BF16 = ml_dtypes.bfloat16

# "f32r": conv2 + gather run in the PE's fast 12-mantissa-bit fp32 mode
#         (measured end-to-end max rel err ~2e-4, ~1.5x faster).
# "exact": everything in bf16 hi/lo 3-term decomposition (max rel ~8e-6).
_MODE = "f32r"

# ---------------- static module config ----------------
_B, _CIN, _H, _W = 4, 3, 256, 256
_BEV_H = _BEV_W = 200
_C1, _C2 = 128, 256
_JH = _BEV_W // 2          # 100 BEV columns per core
_W1COLS = 131              # conv1 output cols per core; local L <-> global wh0-1+L
_IMGCOLS = 133             # image cols per core;        local c <-> global wh0-2+c
_NEG = np.float32(-1e30)


def _compute_static():
    """Static gather maps.  MUST replicate the reference's float32 jax math
    bit-exactly (trunc boundaries differ from float64)."""
    import jax
    import jax.numpy as jnp

    cpu = jax.devices("cpu")[0]
    with jax.default_device(cpu):
        y = jnp.linspace(-50.0, 50.0, _BEV_H)
        x = jnp.linspace(-50.0, 50.0, _BEV_W)
        depth = y / 100.0 * 100.0
        u = jnp.trunc(_W / 2 + x[None, :] / depth[:, None] * 50.0).astype(jnp.int32)
        v = jnp.trunc(_H / 2 - 1.5 / depth * 50.0).astype(jnp.int32)
        mask = (y[:, None] > 0.1) & (u >= 0) & (u < _W) & \
               (v[:, None] >= 0) & (v[:, None] < _H)
    U = np.asarray(u).astype(np.int64)
    V = np.asarray(v).astype(np.int64)
    MASK = np.asarray(mask)

    valid_rows = np.where(MASK.any(axis=1))[0]
    I0 = int(valid_rows.min())
    used_v = sorted(set(int(V[i]) for i in valid_rows))

    # conv1 rows needed (feature rows +-1), as packed list
    need1 = sorted(set(r for vv in used_v for r in (vv - 1, vv, vv + 1)))
    r1_idx = {r: k for k, r in enumerate(need1)}

    # consecutive BEV rows sharing one feature row v
    groups = []
    cur_v = None
    for i in range(I0, _BEV_H):
        vv = int(V[i])
        if cur_v is not None and vv == cur_v:
            groups[-1] = (cur_v, groups[-1][1], groups[-1][2] + 1)
        else:
            groups.append((vv, i, 1))
            cur_v = vv
    v_idx = {vv: k for k, vv in enumerate(used_v)}
    return dict(U=U, V=V, MASK=MASK, I0=I0, USED_V=used_v, R1_ROWS=need1,
                R1_IDX=r1_idx, GROUPS=groups, V_IDX=v_idx)


_ST = None


def _static():
    global _ST
    if _ST is None:
        _ST = _compute_static()
    return _ST


# ---------------- host-side input prep ----------------

def _split_hilo(x):
    x = np.ascontiguousarray(x, dtype=np.float32)
    hi = x.astype(_BF16)
    lo = (x - hi.astype(np.float32)).astype(_BF16)
    return hi, lo


def _build_S(st, jhalf, wh0):
    I0 = st["I0"]
    n_rows = _BEV_H - I0
    S = np.zeros((128, n_rows * _JH), dtype=np.float32)
    U, MASK = st["U"], st["MASK"]
    for i in range(I0, _BEV_H):
        base = (i - I0) * _JH
        for j in range(_JH):
            jj = jhalf * _JH + j
            if MASK[i, jj]:
                S[int(U[i, jj]) - wh0, base + j] = 1.0
    return S


def _host_prep(st, images, fw1, fb1, fw2, fb2):
    """Returns list of 8 in_maps (core = b*2 + jhalf)."""
    R1_ROWS = st["R1_ROWS"]
    nr1 = len(R1_ROWS)
    img_rows = sorted(set(r for r1 in R1_ROWS for r in (r1 - 1, r1, r1 + 1)))
    img_idx = {r: k for k, r in enumerate(img_rows)}

    # weights (shared across cores)
    w1p = np.zeros((27, _C1), np.float32)
    for dy in range(3):
        for dx in range(3):
            for ci in range(3):
                w1p[(dy * 3 + dx) * 3 + ci, :] = fw1[:, ci, dy, dx]
    w1_hi, w1_lo = _split_hilo(w1p)
    fb1h, fb1l = _split_hilo(fb1)
    # stacked conv1 lhsT [84, 128]: [w1_hi; w1_hi; w1_lo; fb1_hi; fb1_lo; ones]
    w1_stack = np.concatenate([
        w1_hi.astype(np.float32), w1_hi.astype(np.float32),
        w1_lo.astype(np.float32),
        fb1h.astype(np.float32)[None, :], fb1l.astype(np.float32)[None, :],
        np.ones((1, _C1), np.float32)]).astype(_BF16)

    w2t = np.zeros((9, _C1, _C2), np.float32)
    for dy in range(3):
        for dx in range(3):
            w2t[dy * 3 + dx] = fw2[:, :, dy, dx].T
    w2km = np.ascontiguousarray(w2t.transpose(1, 0, 2))  # [128, 9, 256]
    if _MODE == "f32r":
        w2_a = w2km
        w2_b = w2km  # unused placeholder (same shape)
        b2_lhsT = np.ones((1, _C1), np.float32)
        b2_rhs = np.ascontiguousarray(fb2[None, :].astype(np.float32))
    else:
        w2_a, w2_b = _split_hilo(w2km)
        fb2h, fb2l = _split_hilo(fb2)
        b2_lhsT = np.ones((2, _C1), np.float32).astype(_BF16)
        b2_rhs = np.ascontiguousarray(np.stack(
            [fb2h.astype(np.float32), fb2l.astype(np.float32)]).astype(_BF16))

    in_maps = []
    S_cache = {}
    for b in range(_B):
        for jh in range(2):
            wh0 = jh * 128
            # padded image slice [3, nimg, 133], global cols wh0-2 .. wh0+130
            imgp = np.zeros((3, len(img_rows), _IMGCOLS), np.float32)
            g0 = wh0 - 2
            lo = max(0, -g0)
            hi = min(_IMGCOLS, _W - g0)
            for k, r in enumerate(img_rows):
                imgp[:, k, lo:hi] = images[b, :, r, g0 + lo:g0 + hi]
            img_hi, img_lo = _split_hilo(imgp)

            # host-side im2col: [27, nr1, 131]
            def _im2col(src):
                G = np.zeros((27, nr1, _W1COLS), src.dtype)
                for rr, r1 in enumerate(R1_ROWS):
                    for dy in range(3):
                        ip = img_idx[r1 + dy - 1]
                        for dx in range(3):
                            for ci in range(3):
                                G[(dy * 3 + dx) * 3 + ci, rr, :] = \
                                    src[ci, ip, dx:dx + _W1COLS]
                return G

            cb = np.zeros((_W1COLS,), np.float32)
            cb[0 if jh == 0 else 129] = _NEG  # zero-pad column of R1 via relu(-inf)
            ones_r = np.ones((nr1, _W1COLS), np.float32)
            # stacked conv1 rhs [84, nr1, 131]: [g1_hi; g1_lo; g1_hi; 1; colbias; 1... ]
            # rows 81..83 pair with lhsT rows [fb1_hi; fb1_lo; ones]:
            #   row81 = ones (adds fb1_hi), row82 = ones (adds fb1_lo),
            #   row83 = colbias (ones row in lhsT -> adds colbias per column)
            g1h = _im2col(img_hi).astype(np.float32)
            g1l = _im2col(img_lo).astype(np.float32)
            g1_stack = np.concatenate([
                g1h, g1l, g1h,
                ones_r[None], ones_r[None],
                np.broadcast_to(cb[None, None, :], (1, nr1, _W1COLS)),
            ]).astype(_BF16)

            if jh not in S_cache:
                s = _build_S(st, jh, wh0)
                S_cache[jh] = (s.astype(np.float32) if _MODE == "f32r"
                               else s.astype(_BF16))

            m = {
                "g1_stack": np.ascontiguousarray(g1_stack),
                "w1_stack": w1_stack,
                "b2_lhsT": b2_lhsT, "b2_rhs": b2_rhs,
                "S": S_cache[jh],
            }
            if _MODE == "f32r":
                m["w2_r"] = w2_a
            else:
                m["w2_hi"] = w2_a
                m["w2_lo"] = w2_b
            in_maps.append(m)
    return in_maps


# ---------------- bass program ----------------

def _build_nc(st):
    import concourse.bass as bass
    import concourse.mybir as mybir
    import concourse.tile as tile
    from concourse import bacc
    from contextlib import ExitStack

    bf = mybir.dt.bfloat16
    f32 = mybir.dt.float32
    f32r = mybir.dt.float32r
    fr = _MODE == "f32r"

    R1_ROWS, R1_IDX = st["R1_ROWS"], st["R1_IDX"]
    USED_V, V_IDX, GROUPS, I0 = st["USED_V"], st["V_IDX"], st["GROUPS"], st["I0"]
    nr1 = len(R1_ROWS)
    nv = len(USED_V)
    n_rows = _BEV_H - I0
    scols = n_rows * _JH

    nc = bacc.Bacc(None, target_bir_lowering=False)

    g1_stack = nc.dram_tensor("g1_stack", [84, nr1, _W1COLS], bf, kind="ExternalInput")
    w1_stack = nc.dram_tensor("w1_stack", [84, _C1], bf, kind="ExternalInput")
    if fr:
        w2_hi = nc.dram_tensor("w2_r", [_C1, 9, _C2], f32r, kind="ExternalInput")
        w2_lo = None
        b2_lhsT = nc.dram_tensor("b2_lhsT", [1, _C1], f32r, kind="ExternalInput")
        b2_rhs = nc.dram_tensor("b2_rhs", [1, _C2], f32r, kind="ExternalInput")
        S = nc.dram_tensor("S", [128, scols], f32r, kind="ExternalInput")
    else:
        w2_hi = nc.dram_tensor("w2_hi", [_C1, 9, _C2], bf, kind="ExternalInput")
        w2_lo = nc.dram_tensor("w2_lo", [_C1, 9, _C2], bf, kind="ExternalInput")
        b2_lhsT = nc.dram_tensor("b2_lhsT", [2, _C1], bf, kind="ExternalInput")
        b2_rhs = nc.dram_tensor("b2_rhs", [2, _C2], bf, kind="ExternalInput")
        S = nc.dram_tensor("S", [128, scols], bf, kind="ExternalInput")
    out = nc.dram_tensor("out", [_C2, _BEV_H, _JH], f32, kind="ExternalOutput")

    with tile.TileContext(nc) as tc:
        with ExitStack() as ctx:
            singles = ctx.enter_context(tc.tile_pool(name="singles", bufs=1))
            tmp = ctx.enter_context(tc.tile_pool(name="tmp", bufs=3))
            stgp = ctx.enter_context(tc.tile_pool(name="stg", bufs=8))

            def load(name, dram, shape, dt, split=1):
                t = singles.tile(shape, dt, tag=name)
                if split == 1:
                    nc.sync.dma_start(out=t[:], in_=dram[:])
                else:
                    n = shape[1]
                    step = (n + split - 1) // split
                    for a in range(0, n, step):
                        b = min(a + step, n)
                        nc.sync.dma_start(out=t[:, a:b], in_=dram[:, a:b])
                return t

            wdt = f32r if fr else bf
            g1_t = load("g1", g1_stack, [84, nr1, _W1COLS], bf, split=4)
            w1_t = load("w1", w1_stack, [84, _C1], bf)
            w2h_t = load("w2h", w2_hi, [_C1, 9, _C2], wdt)
            w2l_t = None if fr else load("w2l", w2_lo, [_C1, 9, _C2], bf)
            b2l_t = load("b2l", b2_lhsT, [1 if fr else 2, _C1], wdt)
            b2r_t = load("b2r", b2_rhs, [1 if fr else 2, _C2], wdt)
            s_t = load("S", S, [128, scols], wdt, split=4)

            if fr:
                r1h = singles.tile([_C1, nr1, _W1COLS], f32r, tag="r1h")
                r1l = None
                fth = singles.tile([128, nv, _C2], f32r, tag="fth")
                ftl = None
            else:
                r1h = singles.tile([_C1, nr1, _W1COLS], bf, tag="r1h")
                r1l = singles.tile([_C1, nr1, _W1COLS], bf, tag="r1l")
                fth = singles.tile([128, nv, _C2], bf, tag="fth")
                ftl = singles.tile([128, nv, _C2], bf, tag="ftl")

            with ExitStack() as cctx:
                p1 = cctx.enter_context(tc.tile_pool(name="p1", bufs=3, space="PSUM"))
                p2 = cctx.enter_context(tc.tile_pool(name="p2", bufs=3, space="PSUM"))

                # ---- conv1: single K=84 stacked matmul per row ----
                for rr in range(nr1):
                    ps = p1.tile([_C1, _W1COLS], f32, tag="ps1")
                    nc.tensor.matmul(ps[:], w1_t[:], g1_t[:, rr, :],
                                     start=True, stop=True)
                    if fr:
                        nc.scalar.activation(r1h[:, rr, :], ps[:],
                                             mybir.ActivationFunctionType.Relu)
                    else:
                        r1row = tmp.tile([_C1, _W1COLS], f32, tag="r1row")
                        nc.scalar.activation(r1row[:], ps[:],
                                             mybir.ActivationFunctionType.Relu)
                        nc.vector.tensor_copy(r1h[:, rr, :], r1row[:])
                        nc.vector.tensor_sub(r1l[:, rr, :], r1row[:], r1h[:, rr, :])

                # ---- conv2 at the 22 needed feature rows ----
                for vi, vv in enumerate(USED_V):
                    ps = p2.tile([128, _C2], f32, tag="ps2")
                    first = True
                    for dy in range(3):
                        rr = R1_IDX[vv + dy - 1]
                        for dx in range(3):
                            tap = dy * 3 + dx
                            lh = r1h[:, rr, dx:dx + 128]
                            rh = w2h_t[:, tap, :]
                            nc.tensor.matmul(ps[:], lh, rh, start=first, stop=False)
                            first = False
                            if not fr:
                                ll = r1l[:, rr, dx:dx + 128]
                                rl = w2l_t[:, tap, :]
                                nc.tensor.matmul(ps[:], ll, rh, start=False, stop=False)
                                nc.tensor.matmul(ps[:], lh, rl, start=False, stop=False)
                    nc.tensor.matmul(ps[:], b2l_t[:], b2r_t[:], start=False, stop=True)
                    if fr:
                        nc.vector.tensor_copy(fth[:, vi, :], ps[:])
                    else:
                        nc.scalar.copy(fth[:, vi, :], ps[:])
                        nc.vector.tensor_sub(ftl[:, vi, :], ps[:], fth[:, vi, :])

            # ---- gather (selection matmuls), 5 BEV rows (=500 cols) per chunk ----
            with ExitStack() as gctx:
                pg = gctx.enter_context(tc.tile_pool(name="pg", bufs=6, space="PSUM"))
                copy_idx = 0
                for (vv, ist, nrow) in GROUPS:
                    vi = V_IDX[vv]
                    done = 0
                    while done < nrow:
                        step = min(5, nrow - done)
                        c0 = (ist - I0 + done) * _JH
                        ncols = step * _JH
                        for ch in range(2):
                            psg = pg.tile([128, 5 * _JH], f32, tag="psg")
                            lh = fth[:, vi, ch * 128:(ch + 1) * 128]
                            if fr:
                                nc.tensor.matmul(psg[:, 0:ncols], lh,
                                                 s_t[:, c0:c0 + ncols],
                                                 start=True, stop=True)
                            else:
                                ll = ftl[:, vi, ch * 128:(ch + 1) * 128]
                                nc.tensor.matmul(psg[:, 0:ncols], lh,
                                                 s_t[:, c0:c0 + ncols],
                                                 start=True, stop=False)
                                nc.tensor.matmul(psg[:, 0:ncols], ll,
                                                 s_t[:, c0:c0 + ncols],
                                                 start=False, stop=True)
                            stg = stgp.tile([128, 5, _JH], f32, tag="stg")
                            dst = stg[:, 0:step, :].rearrange("p a b -> p (a b)")
                            if copy_idx % 2 == 0:
                                nc.vector.tensor_copy(dst, psg[:, 0:ncols])
                            else:
                                nc.scalar.copy(dst, psg[:, 0:ncols])
                            copy_idx += 1
                            nc.sync.dma_start(
                                out=out[ch * 128:(ch + 1) * 128,
                                        ist + done:ist + done + step, :],
                                in_=stg[:, 0:step, :])
                        done += step

    nc.compile()
    return nc


_NC_CACHE = None


def kernel(images, dw1, db1, dw2, db2, fw1, fb1, fw2, fb2):
    global _NC_CACHE
    from concourse.bass_utils import run_bass_kernel_spmd

    images = np.asarray(images, dtype=np.float32)
    fw1 = np.asarray(fw1, dtype=np.float32)
    fb1 = np.asarray(fb1, dtype=np.float32)
    fw2 = np.asarray(fw2, dtype=np.float32)
    fb2 = np.asarray(fb2, dtype=np.float32)

    st = _static()
    in_maps = _host_prep(st, images, fw1, fb1, fw2, fb2)
    if _NC_CACHE is None:
        _NC_CACHE = _build_nc(st)
    nc = _NC_CACHE

    res = run_bass_kernel_spmd(nc, in_maps, core_ids=list(range(8)))

    full = np.zeros((_B, _C2, _BEV_H, _BEV_W), np.float32)
    for b in range(_B):
        for jh in range(2):
            r = res.results[b * 2 + jh]["out"]
            full[b, :, :, jh * _JH:(jh + 1) * _JH] = r
    return full


# revision 18
# speedup vs baseline: 1.7041x; 1.2262x over previous
"""CameraToBEV Trainium2 kernel.

Pipeline (per the reference module): feature head conv(3->128,3x3) -> ReLU ->
conv(128->256,3x3), then a static perspective gather into a [B,256,200,200]
BEV grid.  The depth head is dead code (its output is unused) and is skipped.

Key structural facts exploited (all static, derived from the module config):
  * only 99 BEV rows (101..199) have any valid cell; each BEV row reads from a
    single feature row; only 22 distinct feature rows are ever gathered;
  * per BEV row the gather column map u(j) is static, so the gather is a
    matmul against a static 0/1 selection matrix (zero columns produce the
    masked zeros exactly);
  * all convolution arithmetic is done as bf16 hi/lo decomposed matmuls
    (3 terms), which reproduces fp32 precision to ~1e-5.

Sharding: 8 cores = batch (4) x BEV-column-half (2).  Each core computes
feature columns for its half (128 wide + halo), all 22 feature rows, and the
99x100 BEV quadrant for its (b, half).  SPMD: one program, per-core inputs.
"""

import numpy as np
import ml_dtypes

_BF16 = ml_dtypes.bfloat16

# "f32r": conv2 + gather run in the PE's fast 12-mantissa-bit fp32 mode
#         (measured end-to-end max rel err ~2e-4, ~1.5x faster).
# "exact": everything in bf16 hi/lo 3-term decomposition (max rel ~8e-6).
_MODE = "f32r"

# ---------------- static module config ----------------
_B, _CIN, _H, _W = 4, 3, 256, 256
_BEV_H = _BEV_W = 200
_C1, _C2 = 128, 256
_JH = _BEV_W // 2          # 100 BEV columns per core
_W1COLS = 131              # conv1 output cols per core; local L <-> global wh0-1+L
_IMGCOLS = 133             # image cols per core;        local c <-> global wh0-2+c
_NEG = np.float32(-1e30)


def _compute_static():
    """Static gather maps.  MUST replicate the reference's float32 jax math
    bit-exactly (trunc boundaries differ from float64)."""
    import jax
    import jax.numpy as jnp

    cpu = jax.devices("cpu")[0]
    with jax.default_device(cpu):
        y = jnp.linspace(-50.0, 50.0, _BEV_H)
        x = jnp.linspace(-50.0, 50.0, _BEV_W)
        depth = y / 100.0 * 100.0
        u = jnp.trunc(_W / 2 + x[None, :] / depth[:, None] * 50.0).astype(jnp.int32)
        v = jnp.trunc(_H / 2 - 1.5 / depth * 50.0).astype(jnp.int32)
        mask = (y[:, None] > 0.1) & (u >= 0) & (u < _W) & \
               (v[:, None] >= 0) & (v[:, None] < _H)
    U = np.asarray(u).astype(np.int64)
    V = np.asarray(v).astype(np.int64)
    MASK = np.asarray(mask)

    valid_rows = np.where(MASK.any(axis=1))[0]
    I0 = int(valid_rows.min())
    used_v = sorted(set(int(V[i]) for i in valid_rows))

    # conv1 rows needed (feature rows +-1), as packed list
    need1 = sorted(set(r for vv in used_v for r in (vv - 1, vv, vv + 1)))
    r1_idx = {r: k for k, r in enumerate(need1)}

    # consecutive BEV rows sharing one feature row v
    groups = []
    cur_v = None
    for i in range(I0, _BEV_H):
        vv = int(V[i])
        if cur_v is not None and vv == cur_v:
            groups[-1] = (cur_v, groups[-1][1], groups[-1][2] + 1)
        else:
            groups.append((vv, i, 1))
            cur_v = vv
    v_idx = {vv: k for k, vv in enumerate(used_v)}
    return dict(U=U, V=V, MASK=MASK, I0=I0, USED_V=used_v, R1_ROWS=need1,
                R1_IDX=r1_idx, GROUPS=groups, V_IDX=v_idx)


_ST = None


def _static():
    global _ST
    if _ST is None:
        _ST = _compute_static()
    return _ST


# ---------------- host-side input prep ----------------

def _split_hilo(x):
    x = np.ascontiguousarray(x, dtype=np.float32)
    hi = x.astype(_BF16)
    lo = (x - hi.astype(np.float32)).astype(_BF16)
    return hi, lo


def _build_S(st, jhalf, wh0):
    I0 = st["I0"]
    n_rows = _BEV_H - I0
    S = np.zeros((128, n_rows * _JH), dtype=np.float32)
    U, MASK = st["U"], st["MASK"]
    for i in range(I0, _BEV_H):
        base = (i - I0) * _JH
        for j in range(_JH):
            jj = jhalf * _JH + j
            if MASK[i, jj]:
                S[int(U[i, jj]) - wh0, base + j] = 1.0
    return S


def _host_prep(st, images, fw1, fb1, fw2, fb2):
    """Returns list of 8 in_maps (core = b*2 + jhalf)."""
    R1_ROWS = st["R1_ROWS"]
    nr1 = len(R1_ROWS)
    img_rows = sorted(set(r for r1 in R1_ROWS for r in (r1 - 1, r1, r1 + 1)))
    img_idx = {r: k for k, r in enumerate(img_rows)}

    # weights (shared across cores)
    w1p = np.zeros((27, _C1), np.float32)
    for dy in range(3):
        for dx in range(3):
            for ci in range(3):
                w1p[(dy * 3 + dx) * 3 + ci, :] = fw1[:, ci, dy, dx]
    w1_hi, w1_lo = _split_hilo(w1p)
    fb1h, fb1l = _split_hilo(fb1)
    # stacked conv1 lhsT [84, 128]: [w1_hi; w1_hi; w1_lo; fb1_hi; fb1_lo; ones]
    w1_stack = np.concatenate([
        w1_hi.astype(np.float32), w1_hi.astype(np.float32),
        w1_lo.astype(np.float32),
        fb1h.astype(np.float32)[None, :], fb1l.astype(np.float32)[None, :],
        np.ones((1, _C1), np.float32)]).astype(_BF16)

    w2t = np.zeros((9, _C1, _C2), np.float32)
    for dy in range(3):
        for dx in range(3):
            w2t[dy * 3 + dx] = fw2[:, :, dy, dx].T
    w2km = np.ascontiguousarray(w2t.transpose(1, 0, 2))  # [128, 9, 256]
    if _MODE == "f32r":
        w2_a = w2km
        w2_b = w2km  # unused placeholder (same shape)
        b2_lhsT = np.ones((1, _C1), np.float32)
        b2_rhs = np.ascontiguousarray(fb2[None, :].astype(np.float32))
    else:
        w2_a, w2_b = _split_hilo(w2km)
        fb2h, fb2l = _split_hilo(fb2)
        b2_lhsT = np.ones((2, _C1), np.float32).astype(_BF16)
        b2_rhs = np.ascontiguousarray(np.stack(
            [fb2h.astype(np.float32), fb2l.astype(np.float32)]).astype(_BF16))

    in_maps = []
    S_cache = {}
    for b in range(_B):
        for jh in range(2):
            wh0 = jh * 128
            # padded image slice [3, nimg, 133], global cols wh0-2 .. wh0+130
            imgp = np.zeros((3, len(img_rows), _IMGCOLS), np.float32)
            g0 = wh0 - 2
            lo = max(0, -g0)
            hi = min(_IMGCOLS, _W - g0)
            for k, r in enumerate(img_rows):
                imgp[:, k, lo:hi] = images[b, :, r, g0 + lo:g0 + hi]
            img_hi, img_lo = _split_hilo(imgp)

            # host-side im2col: [27, nr1, 131]
            def _im2col(src):
                G = np.zeros((27, nr1, _W1COLS), src.dtype)
                for rr, r1 in enumerate(R1_ROWS):
                    for dy in range(3):
                        ip = img_idx[r1 + dy - 1]
                        for dx in range(3):
                            for ci in range(3):
                                G[(dy * 3 + dx) * 3 + ci, rr, :] = \
                                    src[ci, ip, dx:dx + _W1COLS]
                return G

            cb = np.zeros((_W1COLS,), np.float32)
            cb[0 if jh == 0 else 129] = _NEG  # zero-pad column of R1 via relu(-inf)
            ones_r = np.ones((nr1, _W1COLS), np.float32)
            # stacked conv1 rhs [84, nr1, 131]: [g1_hi; g1_lo; g1_hi; 1; colbias; 1... ]
            # rows 81..83 pair with lhsT rows [fb1_hi; fb1_lo; ones]:
            #   row81 = ones (adds fb1_hi), row82 = ones (adds fb1_lo),
            #   row83 = colbias (ones row in lhsT -> adds colbias per column)
            g1h = _im2col(img_hi).astype(np.float32)
            g1l = _im2col(img_lo).astype(np.float32)
            g1_stack = np.concatenate([
                g1h, g1l, g1h,
                ones_r[None], ones_r[None],
                np.broadcast_to(cb[None, None, :], (1, nr1, _W1COLS)),
            ]).astype(_BF16)

            if jh not in S_cache:
                s = _build_S(st, jh, wh0)
                S_cache[jh] = (s.astype(np.float32) if _MODE == "f32r"
                               else s.astype(_BF16))

            m = {
                "g1_stack": np.ascontiguousarray(g1_stack),
                "w1_stack": w1_stack,
                "b2_lhsT": b2_lhsT, "b2_rhs": b2_rhs,
                "S": S_cache[jh],
            }
            if _MODE == "f32r":
                m["w2_r"] = w2_a
            else:
                m["w2_hi"] = w2_a
                m["w2_lo"] = w2_b
            in_maps.append(m)
    return in_maps


# ---------------- bass program ----------------

def _build_nc(st):
    import concourse.bass as bass
    import concourse.mybir as mybir
    import concourse.tile as tile
    from concourse import bacc
    from contextlib import ExitStack

    bf = mybir.dt.bfloat16
    f32 = mybir.dt.float32
    f32r = mybir.dt.float32r
    fr = _MODE == "f32r"

    R1_ROWS, R1_IDX = st["R1_ROWS"], st["R1_IDX"]
    USED_V, V_IDX, GROUPS, I0 = st["USED_V"], st["V_IDX"], st["GROUPS"], st["I0"]
    nr1 = len(R1_ROWS)
    nv = len(USED_V)
    n_rows = _BEV_H - I0
    scols = n_rows * _JH

    nc = bacc.Bacc(None, target_bir_lowering=False)

    g1_stack = nc.dram_tensor("g1_stack", [84, nr1, _W1COLS], bf, kind="ExternalInput")
    w1_stack = nc.dram_tensor("w1_stack", [84, _C1], bf, kind="ExternalInput")
    if fr:
        w2_hi = nc.dram_tensor("w2_r", [_C1, 9, _C2], f32r, kind="ExternalInput")
        w2_lo = None
        b2_lhsT = nc.dram_tensor("b2_lhsT", [1, _C1], f32r, kind="ExternalInput")
        b2_rhs = nc.dram_tensor("b2_rhs", [1, _C2], f32r, kind="ExternalInput")
        S = nc.dram_tensor("S", [128, scols], f32r, kind="ExternalInput")
    else:
        w2_hi = nc.dram_tensor("w2_hi", [_C1, 9, _C2], bf, kind="ExternalInput")
        w2_lo = nc.dram_tensor("w2_lo", [_C1, 9, _C2], bf, kind="ExternalInput")
        b2_lhsT = nc.dram_tensor("b2_lhsT", [2, _C1], bf, kind="ExternalInput")
        b2_rhs = nc.dram_tensor("b2_rhs", [2, _C2], bf, kind="ExternalInput")
        S = nc.dram_tensor("S", [128, scols], bf, kind="ExternalInput")
    out = nc.dram_tensor("out", [_C2, _BEV_H, _JH], f32, kind="ExternalOutput")

    with tile.TileContext(nc) as tc:
        with ExitStack() as ctx:
            singles = ctx.enter_context(tc.tile_pool(name="singles", bufs=1))
            tmp = ctx.enter_context(tc.tile_pool(name="tmp", bufs=3))
            stgp = ctx.enter_context(tc.tile_pool(name="stg", bufs=8))

            def load(name, dram, shape, dt, split=1):
                t = singles.tile(shape, dt, tag=name)
                if split == 1:
                    nc.sync.dma_start(out=t[:], in_=dram[:])
                else:
                    n = shape[1]
                    step = (n + split - 1) // split
                    for a in range(0, n, step):
                        b = min(a + step, n)
                        nc.sync.dma_start(out=t[:, a:b], in_=dram[:, a:b])
                return t

            wdt = f32r if fr else bf
            g1_t = load("g1", g1_stack, [84, nr1, _W1COLS], bf, split=4)
            w1_t = load("w1", w1_stack, [84, _C1], bf)
            w2h_t = load("w2h", w2_hi, [_C1, 9, _C2], wdt)
            w2l_t = None if fr else load("w2l", w2_lo, [_C1, 9, _C2], bf)
            b2l_t = load("b2l", b2_lhsT, [1 if fr else 2, _C1], wdt)
            b2r_t = load("b2r", b2_rhs, [1 if fr else 2, _C2], wdt)
            s_t = load("S", S, [128, scols], wdt, split=4)

            if fr:
                r1h = singles.tile([_C1, nr1, _W1COLS], f32r, tag="r1h")
                r1l = None
                fth = singles.tile([128, nv, _C2], f32r, tag="fth")
                ftl = None
            else:
                r1h = singles.tile([_C1, nr1, _W1COLS], bf, tag="r1h")
                r1l = singles.tile([_C1, nr1, _W1COLS], bf, tag="r1l")
                fth = singles.tile([128, nv, _C2], bf, tag="fth")
                ftl = singles.tile([128, nv, _C2], bf, tag="ftl")

            p1 = ctx.enter_context(tc.tile_pool(name="p1", bufs=2, space="PSUM"))
            p2 = ctx.enter_context(tc.tile_pool(name="p2", bufs=2, space="PSUM"))
            pg = ctx.enter_context(tc.tile_pool(name="pg", bufs=4, space="PSUM"))

            # ---- conv1: single K=84 stacked matmul per row ----
            for rr in range(nr1):
                ps = p1.tile([_C1, _W1COLS], f32, tag="ps1")
                nc.tensor.matmul(ps[:], w1_t[:], g1_t[:, rr, :],
                                 start=True, stop=True)
                if fr:
                    nc.scalar.activation(r1h[:, rr, :], ps[:],
                                         mybir.ActivationFunctionType.Relu)
                else:
                    r1row = tmp.tile([_C1, _W1COLS], f32, tag="r1row")
                    nc.scalar.activation(r1row[:], ps[:],
                                         mybir.ActivationFunctionType.Relu)
                    nc.vector.tensor_copy(r1h[:, rr, :], r1row[:])
                    nc.vector.tensor_sub(r1l[:, rr, :], r1row[:], r1h[:, rr, :])

            # ---- per feature row: conv2 then its gather chunks (keeps PE dense) ----
            copy_idx = 0
            for vi, vv in enumerate(USED_V):
                ps = p2.tile([128, _C2], f32, tag="ps2")
                first = True
                for dy in range(3):
                    rr = R1_IDX[vv + dy - 1]
                    for dx in range(3):
                        tap = dy * 3 + dx
                        lh = r1h[:, rr, dx:dx + 128]
                        rh = w2h_t[:, tap, :]
                        nc.tensor.matmul(ps[:], lh, rh, start=first, stop=False)
                        first = False
                        if not fr:
                            ll = r1l[:, rr, dx:dx + 128]
                            rl = w2l_t[:, tap, :]
                            nc.tensor.matmul(ps[:], ll, rh, start=False, stop=False)
                            nc.tensor.matmul(ps[:], lh, rl, start=False, stop=False)
                nc.tensor.matmul(ps[:], b2l_t[:], b2r_t[:], start=False, stop=True)
                if fr:
                    nc.vector.tensor_copy(fth[:, vi, :], ps[:])
                else:
                    nc.scalar.copy(fth[:, vi, :], ps[:])
                    nc.vector.tensor_sub(ftl[:, vi, :], ps[:], fth[:, vi, :])

                # gather for this feature row; same-lhsT chunks consecutive
                gvv, ist, nrow = GROUPS[vi]
                assert gvv == vv
                for ch in range(2):
                    lh = fth[:, vi, ch * 128:(ch + 1) * 128]
                    ll = None if fr else ftl[:, vi, ch * 128:(ch + 1) * 128]
                    done = 0
                    while done < nrow:
                        step = min(5, nrow - done)
                        c0 = (ist - I0 + done) * _JH
                        ncols = step * _JH
                        psg = pg.tile([128, 5 * _JH], f32, tag="psg")
                        if fr:
                            nc.tensor.matmul(psg[:, 0:ncols], lh,
                                             s_t[:, c0:c0 + ncols],
                                             start=True, stop=True)
                        else:
                            nc.tensor.matmul(psg[:, 0:ncols], lh,
                                             s_t[:, c0:c0 + ncols],
                                             start=True, stop=False)
                            nc.tensor.matmul(psg[:, 0:ncols], ll,
                                             s_t[:, c0:c0 + ncols],
                                             start=False, stop=True)
                        stg = stgp.tile([128, 5, _JH], f32, tag="stg")
                        dst = stg[:, 0:step, :].rearrange("p a b -> p (a b)")
                        if copy_idx % 2 == 0:
                            nc.vector.tensor_copy(dst, psg[:, 0:ncols])
                        else:
                            nc.scalar.copy(dst, psg[:, 0:ncols])
                        copy_idx += 1
                        nc.sync.dma_start(
                            out=out[ch * 128:(ch + 1) * 128,
                                    ist + done:ist + done + step, :],
                            in_=stg[:, 0:step, :])
                        done += step

    nc.compile()
    return nc


_NC_CACHE = None


def kernel(images, dw1, db1, dw2, db2, fw1, fb1, fw2, fb2):
    global _NC_CACHE
    from concourse.bass_utils import run_bass_kernel_spmd

    images = np.asarray(images, dtype=np.float32)
    fw1 = np.asarray(fw1, dtype=np.float32)
    fb1 = np.asarray(fb1, dtype=np.float32)
    fw2 = np.asarray(fw2, dtype=np.float32)
    fb2 = np.asarray(fb2, dtype=np.float32)

    st = _static()
    in_maps = _host_prep(st, images, fw1, fb1, fw2, fb2)
    if _NC_CACHE is None:
        _NC_CACHE = _build_nc(st)
    nc = _NC_CACHE

    res = run_bass_kernel_spmd(nc, in_maps, core_ids=list(range(8)))

    full = np.zeros((_B, _C2, _BEV_H, _BEV_W), np.float32)
    for b in range(_B):
        for jh in range(2):
            r = res.results[b * 2 + jh]["out"]
            full[b, :, :, jh * _JH:(jh + 1) * _JH] = r
    return full
